# revision 23
# baseline (speedup 1.0000x reference)
"""Trainium2 Bass kernel for the GRU-GCN cell (nn_GRUCell).

Sharding: 8 NeuronCores.
 - Attention phases (logits/softmax/PV) are BATCH-parallel: each core owns
   4 batches and all 1024 nodes.
 - Weight-gen and the per-node output matmuls are NODE-parallel (128
   nodes/core, all 32 batches).
 - Three AllToAll collectives redistribute xg2 (gate), z*state, and xg2
   (update) between the batch-parallel and node-parallel layouts.
 - exp(logits) is kept resident in SBUF between the gate and update GCNs.
 - LN runs up-front (one Sqrt table load); exp runs on [128,1024] PSUM
   tiles; out-matmul bias is folded in via a tiled-identity matmul and
   sigmoid/tanh run batched over [128,512] blocks.
All matmuls fp16 operands with fp32 PSUM accumulation.
"""

import os
import sys

sys.path.insert(0, "/opt/trn_rl_repo")
import numpy as np

B, N, D = 32, 1024, 64
DI = DO = 64
C = DI + DO          # 128
OG, OU = 2 * DO, DO  # 128, 64
NCORES = 8
NL = N // NCORES     # 128 nodes per core
BL = B // NCORES     # 4 batches per core
NG = NL // 4         # 32 col-pack groups of 4 nodes
EPS = 1e-12

_CACHE = {}
LAST_RESULT = None  # test harness reads timing info from here


def _np_reference(x, state, node_emb, time_emb, gate_w, gate_b, gate_gamma,
                  gate_beta, upd_w, upd_b, upd_gamma, upd_beta):
    """Plain numpy fallback (general layernorm parameters)."""

    def _ln(v, g, b2):
        mu = v.mean(-1, keepdims=True)
        var = ((v - mu) ** 2).mean(-1, keepdims=True)
        return (v - mu) / np.sqrt(var + EPS) * g + b2

    def _gcn(xg, w_pool, b_pool, g, b2):
        emb = _ln(node_emb[None] + time_emb[:, None], g, b2)
        logits = np.einsum("bnd,bmd->bnm", emb, emb, optimize=True)
        a = np.exp(logits - logits.max(-1, keepdims=True))
        a /= a.sum(-1, keepdims=True)
        xg2 = np.einsum("bnm,bmc->bnc", a, xg, optimize=True)
        w = np.einsum("nd,dkio->nkio", node_emb, w_pool, optimize=True)
        bias = time_emb @ b_pool
        return (np.einsum("bni,nio->bno", xg, w[:, 0], optimize=True)
                + np.einsum("bni,nio->bno", xg2, w[:, 1], optimize=True)
                + bias[:, None, :])

    inp = np.concatenate([x, state], -1)
    zr = 1.0 / (1.0 + np.exp(-_gcn(inp, gate_w, gate_b, gate_gamma, gate_beta)))
    z, r = zr[..., :DO], zr[..., DO:]
    cand = np.concatenate([x, z * state], -1)
    hc = np.tanh(_gcn(cand, upd_w, upd_b, upd_gamma, upd_beta))
    return (r * state + (1.0 - r) * hc).astype(np.float32)


def _install_prof_shim():
    """Provide antenv.axon_hooks if absent so trace=True can NTFF-profile."""
    import types

    if "antenv.axon_hooks" in sys.modules:
        return
    try:
        from trn_agent_boot.trn_boot import _ntff_profile_via_ctypes

        hook = _ntff_profile_via_ctypes("/opt/axon/libaxon_pjrt.so")
    except Exception:
        hook = None
    mod = types.ModuleType("antenv.axon_hooks")
    mod.get_axon_ntff_profile_hook = lambda: hook

    def _set(h):
        mod.get_axon_ntff_profile_hook = lambda: h

    mod.set_axon_ntff_profile_hook = _set
    sys.modules["antenv.axon_hooks"] = mod
    try:
        import antenv

        antenv.axon_hooks = mod
    except Exception:
        pass


def _build(debug=False):
    import concourse.bacc as bacc
    import concourse.mybir as mybir
    from concourse.tile import TileContext
    from concourse.masks import make_identity

    F16 = mybir.dt.float16
    F32 = mybir.dt.float32
    AF = mybir.ActivationFunctionType
    ALU = mybir.AluOpType

    if debug:
        nc = bacc.Bacc("TRN2", target_bir_lowering=False, debug=True)
    else:
        nc = bacc.Bacc()

    def pin(name, shape, dt=F16):
        return nc.declare_dram_parameter(name, shape, dt, isOutput=False)

    neT_h = pin("neT_h", [D, N])                # node_emb^T (LN)
    neL2_h = pin("neL2_h", [128, NL])           # local node_emb^T, k-duplicated
    te_h = pin("te_h", [D, BL], F32)            # local time_emb columns
    teT_h = pin("teT_h", [D, B])                # bias matmul lhsT
    gb_h = pin("gb_h", [D, OG])
    ub_h = pin("ub_h", [D, OU])
    i32_h = pin("i32_h", [32, 128])             # tiled identity for bias matmul
    inp_cm_h = pin("inp_cm_h", [128, BL * 8 * C])   # [m,(bb,q,c)] PV lhsT
    inpT_h = pin("inpT_h", [C, B * NL])         # c-major local [x;state]
    st_grp_h = pin("st_grp_h", [128, NG * DO])  # grouped local state
    pg_h = pin("pg_h", [128, OG * C])           # gate pool [64k+d,(o,i)]
    pu_h = pin("pu_h", [128, OU * C])           # upd pool
    h_out = nc.declare_dram_parameter("h_out", [128, NG * DO], F16, isOutput=True)

    with TileContext(nc) as tc:
        with (
            tc.tile_pool(name="const", bufs=1) as cpool,
            tc.tile_pool(name="big", bufs=1) as big,
            tc.tile_pool(name="dram", bufs=1, space="DRAM") as dram,
        ):
            # ---------- constants ----------
            ones64 = cpool.tile([D, D], F16, tag="ones64")
            nc.gpsimd.memset(ones64[:], 1.0)
            ones128 = cpool.tile([128, 128], F16, tag="ones128")
            nc.gpsimd.memset(ones128[:], 1.0)
            ident16 = cpool.tile([128, 128], F16, tag="ident16")
            make_identity(nc, ident16[:])
            eps_col = cpool.tile([D, 1], F32, tag="eps_col")
            nc.gpsimd.memset(eps_col[:], EPS)
            neg64_col = cpool.tile([128, 1], F32, tag="neg64_col")
            nc.gpsimd.memset(neg64_col[:], -64.0)

            # ---------- persistent SBUF ----------
            neT_sb = cpool.tile([D, N], F16, tag="neT_sb")
            nc.sync.dma_start(neT_sb[:], neT_h[:])
            neL_sb = cpool.tile([128, NL], F16, tag="neL_sb")
            nc.sync.dma_start(neL_sb[:], neL2_h[:])
            te_sb = cpool.tile([D, BL], F32, tag="te_sb")
            nc.sync.dma_start(te_sb[:], te_h[:])
            teT_sb = cpool.tile([D, B], F16, tag="teT_sb")
            nc.sync.dma_start(teT_sb[:], teT_h[:])
            gb_sb = cpool.tile([D, OG], F16, tag="gb_sb")
            nc.sync.dma_start(gb_sb[:], gb_h[:])
            ub_sb = cpool.tile([D, OU], F16, tag="ub_sb")
            nc.sync.dma_start(ub_sb[:], ub_h[:])
            i32_sb = cpool.tile([32, 128], F16, tag="i32_sb")
            nc.sync.dma_start(i32_sb[:], i32_h[:])

            exp_sb = big.tile([128, BL * 2 * 4096], F16, tag="exp_sb")
            wslab = big.tile([C, 2 * OG * NL], F16, tag="wslab")
            zr_sb = big.tile([128, NG * OG], F16, tag="zr_sb")
            st_grp = big.tile([128, NG * DO], F16, tag="st_grp")
            nc.sync.dma_start(st_grp[:], st_grp_h[:])
            sinv2 = big.tile([128, BL * 512], F16, tag="sinv2")
            bg4 = big.tile([32, 4 * OG], F16, tag="bg4")
            bu8 = big.tile([32, 8 * OU], F16, tag="bu8")

            # DRAM scratch for the AllToAlls
            d1a_in = dram.tile([NCORES, C, 2, NL], F16, tag="d1a_in")
            d1b_in = dram.tile([NCORES, C, 2, NL], F16, tag="d1b_in")
            d1a_out = dram.tile([NCORES, C, 2, NL], F16, tag="d1a_out")
            d1b_out = dram.tile([NCORES, C, 2, NL], F16, tag="d1b_out")
            d2_in = dram.tile([NCORES, DO, BL, NL], F16, tag="d2_in")
            d2_out = dram.tile([NCORES, DO, BL, NL], F16, tag="d2_out")
            d3_in = dram.tile([NCORES, DO, BL, NL], F16, tag="d3_in")
            d3_out = dram.tile([NCORES, DO, BL, NL], F16, tag="d3_out")

            # ---------- bias rows: time_emb @ pool_b, tiled ----------
            with tc.tile_pool(name="psb", bufs=1, space="PSUM") as psb:
                ps_bg = psb.tile([B, OG], F32, tag="ps_bg")
                nc.tensor.matmul(ps_bg[:], teT_sb[:], gb_sb[:], start=True, stop=True)
                for j in range(4):
                    nc.vector.tensor_copy(bg4[:, j * OG:(j + 1) * OG], ps_bg[:])
                ps_bu = psb.tile([B, OU], F32, tag="ps_bu")
                nc.tensor.matmul(ps_bu[:], teT_sb[:], ub_sb[:], start=True, stop=True)
                for j in range(8):
                    nc.vector.tensor_copy(bu8[:, j * OU:(j + 1) * OU], ps_bu[:])

            # ================= batch-parallel phase =================
            with tc.tile_pool(name="attn_sb", bufs=1) as asb:
                embT = asb.tile([128, BL * N], F16, tag="embT")
                inp_cm = asb.tile([128, BL * 8 * C], F16, tag="inp_cm")
                nc.sync.dma_start(inp_cm[:], inp_cm_h[:])
                xg2T_loc = asb.tile([C, BL * N], F16, tag="xg2T_loc")

                # ---- attention (with LN + gate weight-gen interleaved) ----
                with (
                    tc.tile_pool(name="ln_sb", bufs=2) as lsb,
                    tc.tile_pool(name="ln_stg", bufs=2) as lstg,
                ):
                  with tc.tile_pool(name="ln_ps", bufs=2,
                                    space="PSUM") as lps:
                    def emit_ln(bb):
                        u16 = lsb.tile([D, N], F16, tag="u16")
                        nc.vector.tensor_scalar(
                            out=u16[:], in0=neT_sb[:],
                            scalar1=te_sb[:, bb: bb + 1],
                            scalar2=None, op0=ALU.add,
                        )
                        u2 = lsb.tile([D, N], F16, tag="u2")
                        nc.vector.tensor_mul(u2[:], u16[:], u16[:])
                        for hh in range(2):
                            sl = slice(hh * 512, (hh + 1) * 512)
                            ps_sq = lps.tile([D, 1024], F32, tag="ps_sq")
                            nc.tensor.matmul(ps_sq[0:D, 0:512], ones64[:],
                                             u16[:, sl], start=True, stop=True)
                            nc.tensor.matmul(ps_sq[0:D, 512:1024], ones64[:],
                                             u2[:, sl], start=True, stop=True)
                            mu16 = lstg.tile([D, 512], F16, tag="mu16")
                            nc.vector.tensor_scalar_mul(mu16[:], ps_sq[0:D, 0:512],
                                                        1.0 / D)
                            msq = lstg.tile([D, 512], F16, tag="msq")
                            nc.vector.tensor_mul(msq[:], mu16[:], mu16[:])
                            sd32 = lstg.tile([D, 512], F32, tag="sd32")
                            nc.vector.scalar_tensor_tensor(
                                sd32[:], ps_sq[0:D, 512:1024], 1.0 / D, msq[:],
                                ALU.mult, ALU.subtract)
                            nc.scalar.activation(sd32[:], sd32[:], AF.Sqrt,
                                                 bias=eps_col[:])
                            rinv = lstg.tile([D, 512], F32, tag="rinv")
                            nc.vector.reciprocal_approx_fast(rinv[:], sd32[:])
                            nc.vector.tensor_sub(u16[:, sl], u16[:, sl], mu16[:])
                            dst = embT[0:D, bb * N + hh * 512: bb * N + hh * 512 + 512]
                            nc.vector.tensor_mul(dst, u16[:, sl], rinv[:])
                            nc.vector.tensor_copy(
                                embT[D:128, bb * N + hh * 512:
                                     bb * N + hh * 512 + 512], dst)

                    for bb in range(BL):
                        emit_ln(bb)

                  with (
                    tc.tile_pool(name="astg", bufs=2) as astg,
                    tc.tile_pool(name="plog", bufs=2, space="PSUM") as plog,
                    tc.tile_pool(name="pden", bufs=2, space="PSUM") as pden,
                    tc.tile_pool(name="ppv", bufs=2, space="PSUM") as ppv,
                  ):
                    for bb in range(BL):
                        for ncol in range(2):
                            nb = ncol * 512
                            base = (bb * 2 + ncol) * 4096
                            for t in range(4):
                                ps_l = plog.tile([128, 1024], F32, tag="ps_l")
                                for k2 in range(2):
                                    q = 2 * t + k2
                                    h = (q % 2) * D
                                    nc.tensor.matmul(
                                        ps_l[:, k2 * 512: k2 * 512 + 512],
                                        embT[h: h + D,
                                             bb * N + q * 128: bb * N + q * 128 + 128],
                                        embT[h: h + D, bb * N + nb: bb * N + nb + 512],
                                        start=True, stop=True,
                                    )
                                nc.scalar.activation(
                                    exp_sb[:, base + t * 1024:
                                           base + t * 1024 + 1024],
                                    ps_l[:], AF.Exp, bias=neg64_col[:],
                                )
                            # denominator: ones matmul (replicated rows)
                            ps_den = pden.tile([128, 512], F32, tag="ps_den")
                            for q in range(8):
                                nc.tensor.matmul(
                                    ps_den[:], ones128[:],
                                    exp_sb[:, base + q * 512: base + q * 512 + 512],
                                    start=(q == 0), stop=(q == 7),
                                )
                            # PV: xg2^T[c, n] accumulated over m-chunks
                            ps_xg2 = ppv.tile([C, 512], F32, tag="ps_xg2")
                            for q in range(8):
                                nc.tensor.matmul(
                                    ps_xg2[:],
                                    inp_cm[:, (bb * 8 + q) * C: (bb * 8 + q) * C + C],
                                    exp_sb[:, base + q * 512: base + q * 512 + 512],
                                    start=(q == 0), stop=(q == 7),
                                )
                            sinv32 = astg.tile([128, 512], F32, tag="sinv32")
                            nc.vector.reciprocal_approx_fast(sinv32[:], ps_den[:])
                            nc.vector.tensor_copy(
                                sinv2[ncol * 64: ncol * 64 + 64,
                                      bb * 512: bb * 512 + 512],
                                sinv32[0:64, :])
                            sl = slice(bb * N + nb, bb * N + nb + 512)
                            nc.vector.tensor_mul(
                                xg2T_loc[:, sl], ps_xg2[:], sinv32[:])
                            d1h = d1a_in if bb < 2 else d1b_in
                            nc.gpsimd.dma_start(
                                d1h[:].rearrange("j c bb nn -> c bb j nn")
                                [:, bb % 2, 4 * ncol: 4 * ncol + 4, :],
                                xg2T_loc[:, sl].rearrange(
                                    "c (j nn) -> c j nn", nn=NL),
                            )
                            if bb == 1 and ncol == 1:
                                nc.gpsimd.collective_compute(
                                    "AllToAll", mybir.AluOpType.bypass,
                                    replica_groups=[list(range(NCORES))],
                                    ins=[d1a_in.opt()], outs=[d1a_out.opt()],
                                )
                            if bb == 3 and ncol == 1:
                                nc.gpsimd.collective_compute(
                                    "AllToAll", mybir.AluOpType.bypass,
                                    replica_groups=[list(range(NCORES))],
                                    ins=[d1b_in.opt()], outs=[d1b_out.opt()],
                                )

            # ================= weight-gen (gate) — overlaps AllToAll #1b ====
            def wgen(pool_h, n_och, o_base_k1, wps, wstg):
                # each och covers 16 output cols; A = k0 rows, B = k1 rows
                for och in range(n_och):
                    pw_t = wstg.tile([128, 16 * C], F16, tag="pw_t")
                    nc.sync.dma_start(
                        pw_t[:], pool_h[:, och * 16 * C: (och + 1) * 16 * C])
                    for sub in range(2):
                        ps_wA = wps.tile([128, 1024], F32, tag="wA")
                        ps_wB = wps.tile([128, 1024], F32, tag="wB")
                        for oo in range(8):
                            o_l = sub * 8 + oo
                            osl = slice(o_l * C, o_l * C + C)
                            nc.tensor.matmul(
                                ps_wA[:, oo * NL: (oo + 1) * NL],
                                pw_t[0:64, osl], neL_sb[0:64, :],
                                start=True, stop=True,
                            )
                            nc.tensor.matmul(
                                ps_wB[:, oo * NL: (oo + 1) * NL],
                                pw_t[64:128, osl], neL_sb[64:128, :],
                                start=True, stop=True,
                            )
                        ob = (och * 16 + sub * 8) * NL
                        nc.scalar.activation(
                            wslab[:, ob: ob + 1024], ps_wA[:], AF.Copy)
                        nc.vector.tensor_copy(
                            wslab[:, o_base_k1 * NL + ob:
                                  o_base_k1 * NL + ob + 1024], ps_wB[:])

            # ================= weight-gen (gate) — overlaps AllToAll #1b ====
            with (
                tc.tile_pool(name="wstg_g", bufs=2) as wstg_g,
                tc.tile_pool(name="pswg", bufs=2, space="PSUM") as pswg,
            ):
                wgen(pg_h, 8, OG, pswg, wstg_g)

            # ================= node-parallel phase =================
            with tc.tile_pool(name="out_sb", bufs=1) as osb:
                ioT = osb.tile([C, B * NL], F16, tag="ioT")
                nc.sync.dma_start(ioT[:], inpT_h[:])
                xgT = osb.tile([C, B * NL], F16, tag="xgT")
                xgnv = xgT[:].rearrange("c (j bb nn) -> c j bb nn",
                                        bb=BL, nn=NL)
                nc.sync.dma_start(
                    xgnv[:, :, 0:2, :].rearrange("c j bb nn -> c j (bb nn)"),
                    d1a_out[:].rearrange("j c bb nn -> c j (bb nn)"),
                )
                nc.sync.dma_start(
                    xgnv[:, :, 2:4, :].rearrange("c j bb nn -> c j (bb nn)"),
                    d1b_out[:].rearrange("j c bb nn -> c j (bb nn)"),
                )
                wv = wslab[:].rearrange("c (k o n) -> c k o n", k=2, o=OG)
                io_nb = ioT[:].rearrange("c (b n) -> c n b", n=NL)
                xg_nb = xgT[:].rearrange("c (b n) -> c n b", n=NL)

                # ---- gate out-matmuls (bias via matmul) -> z, r ----
                with (
                    tc.tile_pool(name="ostg", bufs=2) as ostg,
                    tc.tile_pool(name="pog", bufs=3, space="PSUM") as pog,
                    tc.tile_pool(name="ptr", bufs=2, space="PSUM") as ptr,
                ):
                    for blk in range(8):
                        ps_o = pog.tile([128, 512], F32, tag="og")
                        nc.tensor.matmul(ps_o[:], i32_sb[:], bg4[:],
                                         start=True, stop=False,
                                         skip_group_check=True)
                        for gg in range(4):
                            g = blk * 4 + gg
                            for jj in range(4):
                                n_ = g * 4 + jj
                                for ki, src in ((0, io_nb), (1, xg_nb)):
                                    nc.tensor.matmul(
                                        ps_o[32 * jj: 32 * jj + 32,
                                             gg * OG: (gg + 1) * OG],
                                        src[:, n_, :], wv[:, ki, :, n_],
                                        start=False,
                                        stop=(gg == 3 and jj == 3 and ki == 1),
                                        tile_position=(0, 32 * jj),
                                        skip_group_check=True,
                                    )
                        nc.scalar.activation(
                            zr_sb[:, blk * 512: (blk + 1) * 512],
                            ps_o[:], AF.Sigmoid)
                        # z*state for these 4 groups + PE transpose into ioT
                        zs4 = ostg.tile([128, 4 * DO], F16, tag="zs4")
                        nc.vector.tensor_mul(
                            zs4[:].rearrange("p (g o) -> p g o", o=DO),
                            zr_sb[:, blk * 512: (blk + 1) * 512]
                            .rearrange("p (g o) -> p g o", o=OG)[:, :, 0:DO],
                            st_grp[:, blk * 4 * DO: (blk + 1) * 4 * DO]
                            .rearrange("p (g o) -> p g o", o=DO),
                        )
                        ps_t = ptr.tile([DO, 512], F16, tag="ps_t")
                        for gg in range(4):
                            nc.tensor.transpose(
                                ps_t[:, gg * 128: (gg + 1) * 128],
                                zs4[:, gg * DO: (gg + 1) * DO], ident16[:])
                        dst = (ioT[DI:C, :]
                               .rearrange("c (b n) -> c b n", n=NL)
                               [:, :, blk * 16: blk * 16 + 16])
                        src_t = ps_t[:].rearrange(
                            "c (g j b) -> c b (g j)", g=4, j=4)
                        nc.vector.tensor_copy(dst, src_t)

                    # ---- AllToAll #2: (z*state)^T, node-shard -> batch-shard
                    zsv = ioT[DI:C, :].rearrange("o (b n) -> o b n", n=NL)
                    for j in range(NCORES):
                        nc.gpsimd.dma_start(
                            d2_in[j], zsv[:, BL * j: BL * j + BL, :])
                    nc.gpsimd.collective_compute(
                        "AllToAll", mybir.AluOpType.bypass,
                        replica_groups=[list(range(NCORES))],
                        ins=[d2_in.opt()], outs=[d2_out.opt()],
                    )

                # ---- w-gen update pool (reuses wslab low half) ----
                with (
                    tc.tile_pool(name="wstg_u", bufs=2) as wstg_u,
                    tc.tile_pool(name="pswu", bufs=2, space="PSUM") as pswu,
                ):
                    wgen(pu_h, 4, OU, pswu, wstg_u)

                # ---- receive z*state, transpose to column-major ----
                zs_cmT = osb.tile([DO, BL * N], F16, tag="zs_cmT")
                zcv = zs_cmT[:].rearrange("o (bb j nn) -> o bb j nn",
                                          j=NCORES, nn=NL)
                for j in range(NCORES):
                    nc.sync.dma_start(zcv[:, :, j, :], d2_out[j])
                zs_cm = osb.tile([128, BL * 8 * DO], F16, tag="zs_cm")
                hc_sb = osb.tile([128, NG * DO], F16, tag="hc_sb")
                h_sb = osb.tile([128, NG * DO], F16, tag="h_sb")

                with (
                    tc.tile_pool(name="fstg", bufs=1) as fstg,
                    tc.tile_pool(name="ptr2", bufs=2, space="PSUM") as ptr2,
                    tc.tile_pool(name="pxu", bufs=2, space="PSUM") as pxu,
                    tc.tile_pool(name="pou", bufs=1, space="PSUM") as pou,
                ):
                    # ---- update out-matmuls: bias+k0 early (fills A2A gaps)
                    wuv = (wslab[:, : 2 * OU * NL]
                           .rearrange("c (k o n) -> c k o n", k=2, o=OU))
                    ps_us = []
                    for blk in range(4):
                        ps_u = pou.tile([128, 512], F32, tag=f"ou{blk}")
                        ps_us.append(ps_u)
                        nc.tensor.matmul(ps_u[:], i32_sb[:], bu8[:],
                                         start=True, stop=False,
                                         skip_group_check=True)
                        for gg in range(8):
                            g = blk * 8 + gg
                            for jj in range(4):
                                n_ = g * 4 + jj
                                nc.tensor.matmul(
                                    ps_u[32 * jj: 32 * jj + 32,
                                         gg * OU: (gg + 1) * OU],
                                    io_nb[:, n_, :], wuv[:, 0, :, n_],
                                    start=False, stop=False,
                                    tile_position=(0, 32 * jj),
                                    skip_group_check=True,
                                )
                    for ch8 in range(4):
                        ps_t2 = ptr2.tile([128, 512], F16, tag="ps_t2")
                        for k in range(8):
                            chunk = ch8 * 8 + k
                            nc.tensor.transpose(
                                ps_t2[:, k * DO: (k + 1) * DO],
                                zs_cmT[:, chunk * NL: (chunk + 1) * NL],
                                ident16[0:DO, 0:DO])
                        nc.vector.tensor_copy(
                            zs_cm[:, ch8 * 512: (ch8 + 1) * 512], ps_t2[:])

                    # ---- update PV from resident exp ----
                    for bb in range(BL):
                        ps_xu = pxu.tile([128, 512], F32, tag="ps_xu")
                        for ncol in range(2):
                            base = (bb * 2 + ncol) * 4096
                            for q in range(8):
                                nc.tensor.matmul(
                                    ps_xu[64 * ncol: 64 * ncol + 64, :],
                                    zs_cm[:, (bb * 8 + q) * DO:
                                          (bb * 8 + q) * DO + DO],
                                    exp_sb[:, base + q * 512: base + q * 512 + 512],
                                    start=(q == 0), stop=(q == 7),
                                    tile_position=(0, 64 * ncol),
                                )
                        xg2u_bb = fstg.tile([128, 512], F16, tag=f"xgu{bb}")
                        nc.vector.tensor_mul(
                            xg2u_bb[:],
                            ps_xu[:], sinv2[:, bb * 512: (bb + 1) * 512])
                        for ncol in range(2):
                            nc.gpsimd.dma_start(
                                d3_in[4 * ncol: 4 * ncol + 4, :, bb, :]
                                .rearrange("j o nn -> o j nn"),
                                xg2u_bb[64 * ncol: 64 * ncol + 64, :]
                                .rearrange("o (j nn) -> o j nn", nn=NL),
                            )
                    nc.gpsimd.collective_compute(
                        "AllToAll", mybir.AluOpType.bypass,
                        replica_groups=[list(range(NCORES))],
                        ins=[d3_in.opt()], outs=[d3_out.opt()],
                    )

                    # receive xg2 (update) into xgT rows 64:128
                    nc.sync.dma_start(
                        xgT[DI:C, :].rearrange("c (j r) -> c j r", j=NCORES),
                        d3_out[:].rearrange("j c bb nn -> c j (bb nn)"),
                    )
                    for blk in range(4):
                        ps_u = ps_us[blk]
                        for gg in range(8):
                            g = blk * 8 + gg
                            for jj in range(4):
                                n_ = g * 4 + jj
                                nc.tensor.matmul(
                                    ps_u[32 * jj: 32 * jj + 32,
                                         gg * OU: (gg + 1) * OU],
                                    xg_nb[:, n_, :], wuv[:, 1, :, n_],
                                    start=False,
                                    stop=(gg == 7 and jj == 3),
                                    tile_position=(0, 32 * jj),
                                    skip_group_check=True,
                                )
                        nc.scalar.activation(
                            hc_sb[:, blk * 512: (blk + 1) * 512],
                            ps_u[:], AF.Tanh)

                    # ---- final combine: h = hc + r*(state - hc) ----
                    t1 = fstg.tile([128, NG * DO], F16, tag="t1")
                    nc.vector.tensor_sub(t1[:], st_grp[:], hc_sb[:])
                    nc.vector.tensor_mul(
                        t1[:].rearrange("p (g o) -> p g o", o=DO),
                        t1[:].rearrange("p (g o) -> p g o", o=DO),
                        zr_sb[:].rearrange("p (g o) -> p g o", o=OG)[:, :, DO:OG],
                    )
                    nc.vector.tensor_add(h_sb[:], t1[:], hc_sb[:])
                    nc.sync.dma_start(h_out[:], h_sb[:])

    nc.finalize()
    return nc


def _get_nc():
    if "nc" not in _CACHE:
        _CACHE["nc"] = _build()
    return _CACHE["nc"]


def kernel(x, state, node_emb, time_emb, gate_w, gate_b, gate_gamma, gate_beta,
           upd_w, upd_b, upd_gamma, upd_beta):
    global LAST_RESULT
    x = np.asarray(x, np.float32)
    state = np.asarray(state, np.float32)
    node_emb = np.asarray(node_emb, np.float32)
    time_emb = np.asarray(time_emb, np.float32)
    gate_w = np.asarray(gate_w, np.float32)
    gate_b = np.asarray(gate_b, np.float32)
    upd_w = np.asarray(upd_w, np.float32)
    upd_b = np.asarray(upd_b, np.float32)

    shared = (
        np.array_equal(np.asarray(gate_gamma), np.ones(D, np.float32))
        and np.array_equal(np.asarray(upd_gamma), np.ones(D, np.float32))
        and np.array_equal(np.asarray(gate_beta), np.zeros(D, np.float32))
        and np.array_equal(np.asarray(upd_beta), np.zeros(D, np.float32))
    )
    if not shared:
        return _np_reference(x, state, node_emb, time_emb, gate_w, gate_b,
                             gate_gamma, gate_beta, upd_w, upd_b, upd_gamma,
                             upd_beta)

    if os.environ.get("BASS_TRACE"):
        _install_prof_shim()

    from concourse.bass_utils import run_bass_kernel_spmd

    nc = _get_nc()
    in_maps = _prep_inmaps(x, state, node_emb, time_emb, gate_w, gate_b,
                           upd_w, upd_b)

    res = run_bass_kernel_spmd(
        nc, in_maps, list(range(NCORES)),
        trace=bool(os.environ.get("BASS_TRACE")),
    )
    LAST_RESULT = res
    return _unpack_h(lambda c: res.results[c]["h_out"])


def _prep_inmaps(x, state, node_emb, time_emb, gate_w, gate_b, upd_w, upd_b):
    inp = np.concatenate([x, state], -1)                      # [B, N, C]
    inpT = np.ascontiguousarray(inp.transpose(2, 0, 1)).astype(np.float16)
    neT = np.ascontiguousarray(node_emb.T).astype(np.float16)  # [D, N]
    teT = np.ascontiguousarray(time_emb.T).astype(np.float16)  # [D, B]
    pg_h = np.ascontiguousarray(
        gate_w.transpose(1, 0, 3, 2).reshape(128, OG * C)).astype(np.float16)
    pu_h = np.ascontiguousarray(
        upd_w.transpose(1, 0, 3, 2).reshape(128, OU * C)).astype(np.float16)
    i32 = np.ascontiguousarray(np.tile(np.eye(32, dtype=np.float16), (1, 4)))

    in_maps = []
    for c in range(NCORES):
        nsl = slice(c * NL, (c + 1) * NL)
        bsl = slice(c * BL, (c + 1) * BL)
        ne2 = np.empty((128, NL), np.float16)
        ne2[0:64] = neT[:, nsl]
        ne2[64:128] = neT[:, nsl]
        inp_cm = np.ascontiguousarray(
            inp[bsl].reshape(BL, 8, 128, C).transpose(2, 0, 1, 3)
            .reshape(128, BL * 8 * C)).astype(np.float16)
        st_grp = np.ascontiguousarray(
            state[:, nsl, :].reshape(B, NG, 4, DO).transpose(2, 0, 1, 3)
            .reshape(128, NG * DO)).astype(np.float16)
        in_maps.append({
            "neT_h": neT,
            "neL2_h": ne2,
            "te_h": np.ascontiguousarray(time_emb.T[:, bsl]).astype(np.float32),
            "teT_h": teT,
            "gb_h": gate_b.astype(np.float16),
            "ub_h": upd_b.astype(np.float16),
            "i32_h": i32,
            "inp_cm_h": inp_cm,
            "inpT_h": np.ascontiguousarray(inpT[:, :, nsl]).reshape(C, B * NL),
            "st_grp_h": st_grp,
            "pg_h": pg_h,
            "pu_h": pu_h,
        })
    return in_maps


def _unpack_h(get_out):
    h = np.empty((B, N, DO), np.float32)
    for c in range(NCORES):
        ho = get_out(c).astype(np.float32).reshape(4, 32, NG, DO)  # [jj,b,g,o]
        h[:, c * NL: (c + 1) * NL, :] = (
            ho.transpose(1, 2, 0, 3).reshape(B, NL, DO)
        )
    return h


# revision 26
# speedup vs baseline: 1.0738x; 1.0738x over previous
"""Trainium2 Bass kernel for the GRU-GCN cell (nn_GRUCell).

Sharding: 8 NeuronCores.
 - Attention phases (logits/softmax/PV) are BATCH-parallel: each core owns
   4 batches and all 1024 nodes.
 - Weight-gen and the per-node output matmuls are NODE-parallel (128
   nodes/core, all 32 batches).
 - Three AllToAll collectives redistribute xg2 (gate), z*state, and xg2
   (update) between the batch-parallel and node-parallel layouts.
 - exp(logits) is kept resident in SBUF between the gate and update GCNs.
 - LN runs up-front (one Sqrt table load); exp runs on [128,1024] PSUM
   tiles; out-matmul bias is folded in via a tiled-identity matmul and
   sigmoid/tanh run batched over [128,512] blocks.
All matmuls fp16 operands with fp32 PSUM accumulation.
"""

import os
import sys

sys.path.insert(0, "/opt/trn_rl_repo")
import numpy as np

B, N, D = 32, 1024, 64
DI = DO = 64
C = DI + DO          # 128
OG, OU = 2 * DO, DO  # 128, 64
NCORES = 8
NL = N // NCORES     # 128 nodes per core
BL = B // NCORES     # 4 batches per core
NG = NL // 4         # 32 col-pack groups of 4 nodes
EPS = 1e-12

_CACHE = {}
LAST_RESULT = None  # test harness reads timing info from here


def _np_reference(x, state, node_emb, time_emb, gate_w, gate_b, gate_gamma,
                  gate_beta, upd_w, upd_b, upd_gamma, upd_beta):
    """Plain numpy fallback (general layernorm parameters)."""

    def _ln(v, g, b2):
        mu = v.mean(-1, keepdims=True)
        var = ((v - mu) ** 2).mean(-1, keepdims=True)
        return (v - mu) / np.sqrt(var + EPS) * g + b2

    def _gcn(xg, w_pool, b_pool, g, b2):
        emb = _ln(node_emb[None] + time_emb[:, None], g, b2)
        logits = np.einsum("bnd,bmd->bnm", emb, emb, optimize=True)
        a = np.exp(logits - logits.max(-1, keepdims=True))
        a /= a.sum(-1, keepdims=True)
        xg2 = np.einsum("bnm,bmc->bnc", a, xg, optimize=True)
        w = np.einsum("nd,dkio->nkio", node_emb, w_pool, optimize=True)
        bias = time_emb @ b_pool
        return (np.einsum("bni,nio->bno", xg, w[:, 0], optimize=True)
                + np.einsum("bni,nio->bno", xg2, w[:, 1], optimize=True)
                + bias[:, None, :])

    inp = np.concatenate([x, state], -1)
    zr = 1.0 / (1.0 + np.exp(-_gcn(inp, gate_w, gate_b, gate_gamma, gate_beta)))
    z, r = zr[..., :DO], zr[..., DO:]
    cand = np.concatenate([x, z * state], -1)
    hc = np.tanh(_gcn(cand, upd_w, upd_b, upd_gamma, upd_beta))
    return (r * state + (1.0 - r) * hc).astype(np.float32)


def _install_prof_shim():
    """Provide antenv.axon_hooks if absent so trace=True can NTFF-profile."""
    import types

    if "antenv.axon_hooks" in sys.modules:
        return
    try:
        from trn_agent_boot.trn_boot import _ntff_profile_via_ctypes

        hook = _ntff_profile_via_ctypes("/opt/axon/libaxon_pjrt.so")
    except Exception:
        hook = None
    mod = types.ModuleType("antenv.axon_hooks")
    mod.get_axon_ntff_profile_hook = lambda: hook

    def _set(h):
        mod.get_axon_ntff_profile_hook = lambda: h

    mod.set_axon_ntff_profile_hook = _set
    sys.modules["antenv.axon_hooks"] = mod
    try:
        import antenv

        antenv.axon_hooks = mod
    except Exception:
        pass


def _build(debug=False):
    import concourse.bacc as bacc
    import concourse.mybir as mybir
    from concourse.tile import TileContext
    from concourse.masks import make_identity

    F16 = mybir.dt.float16
    F32 = mybir.dt.float32
    AF = mybir.ActivationFunctionType
    ALU = mybir.AluOpType

    if debug:
        nc = bacc.Bacc("TRN2", target_bir_lowering=False, debug=True)
    else:
        nc = bacc.Bacc()

    def pin(name, shape, dt=F16):
        return nc.declare_dram_parameter(name, shape, dt, isOutput=False)

    neT_h = pin("neT_h", [D, N])                # node_emb^T (LN)
    neL2_h = pin("neL2_h", [128, NL])           # local node_emb^T, k-duplicated
    te_h = pin("te_h", [D, BL], F32)            # local time_emb columns
    teT_h = pin("teT_h", [D, B])                # bias matmul lhsT
    gb_h = pin("gb_h", [D, OG])
    ub_h = pin("ub_h", [D, OU])
    i32_h = pin("i32_h", [32, 128])             # tiled identity for bias matmul
    inp_cm_h = pin("inp_cm_h", [128, BL * 8 * C])   # [m,(bb,q,c)] PV lhsT
    inpT_h = pin("inpT_h", [C, B * NL])         # c-major local [x;state]
    st_grp_h = pin("st_grp_h", [128, NG * DO])  # grouped local state
    pg_h = pin("pg_h", [128, OG * C])           # gate pool [64k+d,(o,i)]
    pu_h = pin("pu_h", [128, OU * C])           # upd pool
    h_out = nc.declare_dram_parameter("h_out", [128, NG * DO], F16, isOutput=True)

    with TileContext(nc) as tc:
        with (
            tc.tile_pool(name="const", bufs=1) as cpool,
            tc.tile_pool(name="big", bufs=1) as big,
            tc.tile_pool(name="dram", bufs=1, space="DRAM") as dram,
        ):
            # ---------- constants ----------
            ones64 = cpool.tile([D, D], F16, tag="ones64")
            nc.gpsimd.memset(ones64[:], 1.0)
            ones128 = cpool.tile([128, 128], F16, tag="ones128")
            nc.gpsimd.memset(ones128[:], 1.0)
            ident16 = cpool.tile([128, 128], F16, tag="ident16")
            make_identity(nc, ident16[:])
            eps_col = cpool.tile([D, 1], F32, tag="eps_col")
            nc.gpsimd.memset(eps_col[:], EPS)
            neg64_col = cpool.tile([128, 1], F32, tag="neg64_col")
            nc.gpsimd.memset(neg64_col[:], -64.0)

            # ---------- persistent SBUF ----------
            neT_sb = cpool.tile([D, N], F16, tag="neT_sb")
            nc.sync.dma_start(neT_sb[:], neT_h[:])
            neL_sb = cpool.tile([128, NL], F16, tag="neL_sb")
            nc.sync.dma_start(neL_sb[:], neL2_h[:])
            te_sb = cpool.tile([D, BL], F32, tag="te_sb")
            nc.sync.dma_start(te_sb[:], te_h[:])
            teT_sb = cpool.tile([D, B], F16, tag="teT_sb")
            nc.sync.dma_start(teT_sb[:], teT_h[:])
            gb_sb = cpool.tile([D, OG], F16, tag="gb_sb")
            nc.sync.dma_start(gb_sb[:], gb_h[:])
            ub_sb = cpool.tile([D, OU], F16, tag="ub_sb")
            nc.sync.dma_start(ub_sb[:], ub_h[:])
            i32_sb = cpool.tile([32, 128], F16, tag="i32_sb")
            nc.sync.dma_start(i32_sb[:], i32_h[:])

            exp_sb = big.tile([128, BL * 2 * 4096], F16, tag="exp_sb")
            wslab = big.tile([C, 2 * OG * NL], F16, tag="wslab")
            zr_sb = big.tile([128, NG * OG], F16, tag="zr_sb")
            st_grp = big.tile([128, NG * DO], F16, tag="st_grp")
            nc.sync.dma_start(st_grp[:], st_grp_h[:])
            sinv2 = big.tile([128, BL * 512], F16, tag="sinv2")
            bg4 = big.tile([32, 4 * OG], F16, tag="bg4")
            bu8 = big.tile([32, 8 * OU], F16, tag="bu8")

            # DRAM scratch for the AllToAlls
            d1a_in = dram.tile([NCORES, C, 2, NL], F16, tag="d1a_in")
            d1b_in = dram.tile([NCORES, C, 2, NL], F16, tag="d1b_in")
            d1a_out = dram.tile([NCORES, C, 2, NL], F16, tag="d1a_out")
            d1b_out = dram.tile([NCORES, C, 2, NL], F16, tag="d1b_out")
            d2_in = dram.tile([NCORES, DO, BL, NL], F16, tag="d2_in")
            d2_out = dram.tile([NCORES, DO, BL, NL], F16, tag="d2_out")
            d3_in = dram.tile([NCORES, DO, BL, NL], F16, tag="d3_in")
            d3_out = dram.tile([NCORES, DO, BL, NL], F16, tag="d3_out")

            # ---------- bias rows: time_emb @ pool_b, tiled ----------
            with tc.tile_pool(name="psb", bufs=1, space="PSUM") as psb:
                ps_bg = psb.tile([B, OG], F32, tag="ps_bg")
                nc.tensor.matmul(ps_bg[:], teT_sb[:], gb_sb[:], start=True, stop=True)
                for j in range(4):
                    nc.vector.tensor_copy(bg4[:, j * OG:(j + 1) * OG], ps_bg[:])
                ps_bu = psb.tile([B, OU], F32, tag="ps_bu")
                nc.tensor.matmul(ps_bu[:], teT_sb[:], ub_sb[:], start=True, stop=True)
                for j in range(8):
                    nc.vector.tensor_copy(bu8[:, j * OU:(j + 1) * OU], ps_bu[:])

            # ================= batch-parallel phase =================
            with tc.tile_pool(name="attn_sb", bufs=1) as asb:
                embT = asb.tile([128, BL * N], F16, tag="embT")
                inp_cm = asb.tile([128, BL * 8 * C], F16, tag="inp_cm")
                nc.sync.dma_start(inp_cm[:], inp_cm_h[:])
                xg2T_loc = asb.tile([C, BL * N], F16, tag="xg2T_loc")

                # ---- attention (with LN + gate weight-gen interleaved) ----
                with (
                    tc.tile_pool(name="ln_sb", bufs=2) as lsb,
                    tc.tile_pool(name="ln_stg", bufs=2) as lstg,
                    tc.tile_pool(name="ln_ps", bufs=2, space="PSUM") as lps,
                ):
                    def emit_ln(bb):
                        u16 = lsb.tile([D, N], F16, tag="u16")
                        nc.vector.tensor_scalar(
                            out=u16[:], in0=neT_sb[:],
                            scalar1=te_sb[:, bb: bb + 1],
                            scalar2=None, op0=ALU.add,
                        )
                        u2 = lsb.tile([D, N], F16, tag="u2")
                        nc.vector.tensor_mul(u2[:], u16[:], u16[:])
                        ps_sq = lps.tile([D, 2048], F32, tag="ps_sq")
                        for hh in range(2):
                            sl = slice(hh * 512, (hh + 1) * 512)
                            nc.tensor.matmul(ps_sq[0:D, hh * 512: hh * 512 + 512],
                                             ones64[:], u16[:, sl],
                                             start=True, stop=True)
                            nc.tensor.matmul(ps_sq[0:D, 1024 + hh * 512:
                                                   1024 + hh * 512 + 512],
                                             ones64[:], u2[:, sl],
                                             start=True, stop=True)
                        mu16 = lstg.tile([D, N], F16, tag="mu16")
                        nc.vector.tensor_scalar_mul(mu16[:], ps_sq[0:D, 0:1024],
                                                    1.0 / D)
                        msq = lstg.tile([D, N], F16, tag="msq")
                        nc.vector.tensor_mul(msq[:], mu16[:], mu16[:])
                        sd32 = lstg.tile([D, N], F32, tag="sd32")
                        nc.vector.scalar_tensor_tensor(
                            sd32[:], ps_sq[0:D, 1024:2048], 1.0 / D, msq[:],
                            ALU.mult, ALU.subtract)
                        nc.scalar.activation(sd32[:], sd32[:], AF.Sqrt,
                                             bias=eps_col[:])
                        rinv = lstg.tile([D, N], F32, tag="rinv")
                        nc.vector.reciprocal_approx_fast(rinv[:], sd32[:])
                        nc.vector.tensor_sub(u16[:], u16[:], mu16[:])
                        dst = embT[0:D, bb * N: bb * N + N]
                        nc.vector.tensor_mul(dst, u16[:], rinv[:])
                        nc.vector.tensor_copy(embT[D:128, bb * N: bb * N + N],
                                              dst)

                    for bb in range(BL):
                        emit_ln(bb)

                with (
                    tc.tile_pool(name="astg", bufs=2) as astg,
                    tc.tile_pool(name="plog", bufs=2, space="PSUM") as plog,
                    tc.tile_pool(name="pden", bufs=2, space="PSUM") as pden,
                    tc.tile_pool(name="ppv", bufs=2, space="PSUM") as ppv,
                ):
                    for bb in range(BL):
                        for ncol in range(2):
                            nb = ncol * 512
                            base = (bb * 2 + ncol) * 4096
                            for t in range(4):
                                ps_l = plog.tile([128, 1024], F32, tag="ps_l")
                                for k2 in range(2):
                                    q = 2 * t + k2
                                    h = (q % 2) * D
                                    nc.tensor.matmul(
                                        ps_l[:, k2 * 512: k2 * 512 + 512],
                                        embT[h: h + D,
                                             bb * N + q * 128: bb * N + q * 128 + 128],
                                        embT[h: h + D, bb * N + nb: bb * N + nb + 512],
                                        start=True, stop=True,
                                    )
                                nc.scalar.activation(
                                    exp_sb[:, base + t * 1024:
                                           base + t * 1024 + 1024],
                                    ps_l[:], AF.Exp, bias=neg64_col[:],
                                )
                            # denominator: ones matmul (replicated rows)
                            ps_den = pden.tile([128, 512], F32, tag="ps_den")
                            for q in range(8):
                                nc.tensor.matmul(
                                    ps_den[:], ones128[:],
                                    exp_sb[:, base + q * 512: base + q * 512 + 512],
                                    start=(q == 0), stop=(q == 7),
                                )
                            # PV: xg2^T[c, n] accumulated over m-chunks
                            ps_xg2 = ppv.tile([C, 512], F32, tag="ps_xg2")
                            for q in range(8):
                                nc.tensor.matmul(
                                    ps_xg2[:],
                                    inp_cm[:, (bb * 8 + q) * C: (bb * 8 + q) * C + C],
                                    exp_sb[:, base + q * 512: base + q * 512 + 512],
                                    start=(q == 0), stop=(q == 7),
                                )
                            sinv32 = astg.tile([128, 512], F32, tag="sinv32")
                            nc.vector.reciprocal_approx_fast(sinv32[:], ps_den[:])
                            nc.vector.tensor_copy(
                                sinv2[ncol * 64: ncol * 64 + 64,
                                      bb * 512: bb * 512 + 512],
                                sinv32[0:64, :])
                            sl = slice(bb * N + nb, bb * N + nb + 512)
                            nc.vector.tensor_mul(
                                xg2T_loc[:, sl], ps_xg2[:], sinv32[:])
                            d1h = d1a_in if bb < 2 else d1b_in
                            nc.gpsimd.dma_start(
                                d1h[:].rearrange("j c bb nn -> c bb j nn")
                                [:, bb % 2, 4 * ncol: 4 * ncol + 4, :],
                                xg2T_loc[:, sl].rearrange(
                                    "c (j nn) -> c j nn", nn=NL),
                            )
                            if bb == 1 and ncol == 1:
                                nc.gpsimd.collective_compute(
                                    "AllToAll", mybir.AluOpType.bypass,
                                    replica_groups=[list(range(NCORES))],
                                    ins=[d1a_in.opt()], outs=[d1a_out.opt()],
                                )
                            if bb == 3 and ncol == 1:
                                nc.gpsimd.collective_compute(
                                    "AllToAll", mybir.AluOpType.bypass,
                                    replica_groups=[list(range(NCORES))],
                                    ins=[d1b_in.opt()], outs=[d1b_out.opt()],
                                )

            # ================= weight-gen (gate) — overlaps AllToAll #1b ====
            def wgen(pool_h, n_och, o_base_k1, wps, wstg):
                # each och covers 16 output cols; A = k0 rows, B = k1 rows
                for och in range(n_och):
                    pw_t = wstg.tile([128, 16 * C], F16, tag="pw_t")
                    nc.sync.dma_start(
                        pw_t[:], pool_h[:, och * 16 * C: (och + 1) * 16 * C])
                    for sub in range(2):
                        ps_wA = wps.tile([128, 1024], F32, tag="wA")
                        ps_wB = wps.tile([128, 1024], F32, tag="wB")
                        for oo in range(8):
                            o_l = sub * 8 + oo
                            osl = slice(o_l * C, o_l * C + C)
                            nc.tensor.matmul(
                                ps_wA[:, oo * NL: (oo + 1) * NL],
                                pw_t[0:64, osl], neL_sb[0:64, :],
                                start=True, stop=True,
                            )
                            nc.tensor.matmul(
                                ps_wB[:, oo * NL: (oo + 1) * NL],
                                pw_t[64:128, osl], neL_sb[64:128, :],
                                start=True, stop=True,
                            )
                        ob = (och * 16 + sub * 8) * NL
                        nc.scalar.activation(
                            wslab[:, ob: ob + 1024], ps_wA[:], AF.Copy)
                        nc.vector.tensor_copy(
                            wslab[:, o_base_k1 * NL + ob:
                                  o_base_k1 * NL + ob + 1024], ps_wB[:])

            # ================= weight-gen (gate) — overlaps AllToAll #1b ====
            with (
                tc.tile_pool(name="wstg_g", bufs=2) as wstg_g,
                tc.tile_pool(name="pswg", bufs=2, space="PSUM") as pswg,
            ):
                wgen(pg_h, 8, OG, pswg, wstg_g)

            # ================= node-parallel phase =================
            with tc.tile_pool(name="out_sb", bufs=1) as osb:
                ioT = osb.tile([C, B * NL], F16, tag="ioT")
                nc.sync.dma_start(ioT[:], inpT_h[:])
                xgT = osb.tile([C, B * NL], F16, tag="xgT")
                xgnv = xgT[:].rearrange("c (j bb nn) -> c j bb nn",
                                        bb=BL, nn=NL)
                nc.sync.dma_start(
                    xgnv[:, :, 0:2, :].rearrange("c j bb nn -> c j (bb nn)"),
                    d1a_out[:].rearrange("j c bb nn -> c j (bb nn)"),
                )
                nc.sync.dma_start(
                    xgnv[:, :, 2:4, :].rearrange("c j bb nn -> c j (bb nn)"),
                    d1b_out[:].rearrange("j c bb nn -> c j (bb nn)"),
                )
                wv = wslab[:].rearrange("c (k o n) -> c k o n", k=2, o=OG)
                io_nb = ioT[:].rearrange("c (b n) -> c n b", n=NL)
                xg_nb = xgT[:].rearrange("c (b n) -> c n b", n=NL)

                # ---- gate out-matmuls (bias via matmul) -> z, r ----
                with (
                    tc.tile_pool(name="ostg", bufs=2) as ostg,
                    tc.tile_pool(name="pog", bufs=3, space="PSUM") as pog,
                    tc.tile_pool(name="ptr", bufs=2, space="PSUM") as ptr,
                ):
                    for blk in range(8):
                        ps_o = pog.tile([128, 512], F32, tag="og")
                        nc.tensor.matmul(ps_o[:], i32_sb[:], bg4[:],
                                         start=True, stop=False,
                                         skip_group_check=True)
                        for gg in range(4):
                            g = blk * 4 + gg
                            for jj in range(4):
                                n_ = g * 4 + jj
                                for ki, src in ((0, io_nb), (1, xg_nb)):
                                    nc.tensor.matmul(
                                        ps_o[32 * jj: 32 * jj + 32,
                                             gg * OG: (gg + 1) * OG],
                                        src[:, n_, :], wv[:, ki, :, n_],
                                        start=False,
                                        stop=(gg == 3 and jj == 3 and ki == 1),
                                        tile_position=(0, 32 * jj),
                                        skip_group_check=True,
                                    )
                        nc.scalar.activation(
                            zr_sb[:, blk * 512: (blk + 1) * 512],
                            ps_o[:], AF.Sigmoid)
                        # z*state for these 4 groups + PE transpose into ioT
                        zs4 = ostg.tile([128, 4 * DO], F16, tag="zs4")
                        nc.vector.tensor_mul(
                            zs4[:].rearrange("p (g o) -> p g o", o=DO),
                            zr_sb[:, blk * 512: (blk + 1) * 512]
                            .rearrange("p (g o) -> p g o", o=OG)[:, :, 0:DO],
                            st_grp[:, blk * 4 * DO: (blk + 1) * 4 * DO]
                            .rearrange("p (g o) -> p g o", o=DO),
                        )
                        ps_t = ptr.tile([DO, 512], F16, tag="ps_t")
                        for gg in range(4):
                            nc.tensor.transpose(
                                ps_t[:, gg * 128: (gg + 1) * 128],
                                zs4[:, gg * DO: (gg + 1) * DO], ident16[:])
                        dst = (ioT[DI:C, :]
                               .rearrange("c (b n) -> c b n", n=NL)
                               [:, :, blk * 16: blk * 16 + 16])
                        src_t = ps_t[:].rearrange(
                            "c (g j b) -> c b (g j)", g=4, j=4)
                        nc.vector.tensor_copy(dst, src_t)

                    # ---- AllToAll #2: (z*state)^T, node-shard -> batch-shard
                    zsv = ioT[DI:C, :].rearrange("o (b n) -> o b n", n=NL)
                    for j in range(NCORES):
                        nc.gpsimd.dma_start(
                            d2_in[j], zsv[:, BL * j: BL * j + BL, :])
                    nc.gpsimd.collective_compute(
                        "AllToAll", mybir.AluOpType.bypass,
                        replica_groups=[list(range(NCORES))],
                        ins=[d2_in.opt()], outs=[d2_out.opt()],
                    )

                # ---- w-gen update pool (reuses wslab low half) ----
                with (
                    tc.tile_pool(name="wstg_u", bufs=2) as wstg_u,
                    tc.tile_pool(name="pswu", bufs=2, space="PSUM") as pswu,
                ):
                    wgen(pu_h, 4, OU, pswu, wstg_u)

                # ---- receive z*state, transpose to column-major ----
                zs_cmT = osb.tile([DO, BL * N], F16, tag="zs_cmT")
                zcv = zs_cmT[:].rearrange("o (bb j nn) -> o bb j nn",
                                          j=NCORES, nn=NL)
                for j in range(NCORES):
                    nc.sync.dma_start(zcv[:, :, j, :], d2_out[j])
                zs_cm = osb.tile([128, BL * 8 * DO], F16, tag="zs_cm")
                hc_sb = osb.tile([128, NG * DO], F16, tag="hc_sb")
                h_sb = osb.tile([128, NG * DO], F16, tag="h_sb")

                with (
                    tc.tile_pool(name="fstg", bufs=1) as fstg,
                    tc.tile_pool(name="ptr2", bufs=2, space="PSUM") as ptr2,
                    tc.tile_pool(name="pxu", bufs=2, space="PSUM") as pxu,
                    tc.tile_pool(name="pou", bufs=1, space="PSUM") as pou,
                ):
                    # ---- update out-matmuls: bias+k0 early (fills A2A gaps)
                    wuv = (wslab[:, : 2 * OU * NL]
                           .rearrange("c (k o n) -> c k o n", k=2, o=OU))
                    ps_us = []
                    for blk in range(4):
                        ps_u = pou.tile([128, 512], F32, tag=f"ou{blk}")
                        ps_us.append(ps_u)
                        nc.tensor.matmul(ps_u[:], i32_sb[:], bu8[:],
                                         start=True, stop=False,
                                         skip_group_check=True)
                        for gg in range(8):
                            g = blk * 8 + gg
                            for jj in range(4):
                                n_ = g * 4 + jj
                                nc.tensor.matmul(
                                    ps_u[32 * jj: 32 * jj + 32,
                                         gg * OU: (gg + 1) * OU],
                                    io_nb[:, n_, :], wuv[:, 0, :, n_],
                                    start=False, stop=False,
                                    tile_position=(0, 32 * jj),
                                    skip_group_check=True,
                                )
                    for ch8 in range(4):
                        ps_t2 = ptr2.tile([128, 512], F16, tag="ps_t2")
                        for k in range(8):
                            chunk = ch8 * 8 + k
                            nc.tensor.transpose(
                                ps_t2[:, k * DO: (k + 1) * DO],
                                zs_cmT[:, chunk * NL: (chunk + 1) * NL],
                                ident16[0:DO, 0:DO])
                        nc.vector.tensor_copy(
                            zs_cm[:, ch8 * 512: (ch8 + 1) * 512], ps_t2[:])

                    # ---- update PV from resident exp ----
                    for bb in range(BL):
                        ps_xu = pxu.tile([128, 512], F32, tag="ps_xu")
                        for ncol in range(2):
                            base = (bb * 2 + ncol) * 4096
                            for q in range(8):
                                nc.tensor.matmul(
                                    ps_xu[64 * ncol: 64 * ncol + 64, :],
                                    zs_cm[:, (bb * 8 + q) * DO:
                                          (bb * 8 + q) * DO + DO],
                                    exp_sb[:, base + q * 512: base + q * 512 + 512],
                                    start=(q == 0), stop=(q == 7),
                                    tile_position=(0, 64 * ncol),
                                )
                        xg2u_bb = fstg.tile([128, 512], F16, tag=f"xgu{bb}")
                        nc.vector.tensor_mul(
                            xg2u_bb[:],
                            ps_xu[:], sinv2[:, bb * 512: (bb + 1) * 512])
                        for ncol in range(2):
                            nc.gpsimd.dma_start(
                                d3_in[4 * ncol: 4 * ncol + 4, :, bb, :]
                                .rearrange("j o nn -> o j nn"),
                                xg2u_bb[64 * ncol: 64 * ncol + 64, :]
                                .rearrange("o (j nn) -> o j nn", nn=NL),
                            )
                    nc.gpsimd.collective_compute(
                        "AllToAll", mybir.AluOpType.bypass,
                        replica_groups=[list(range(NCORES))],
                        ins=[d3_in.opt()], outs=[d3_out.opt()],
                    )

                    # receive xg2 (update) into xgT rows 64:128
                    nc.sync.dma_start(
                        xgT[DI:C, :].rearrange("c (j r) -> c j r", j=NCORES),
                        d3_out[:].rearrange("j c bb nn -> c j (bb nn)"),
                    )
                    for blk in range(4):
                        ps_u = ps_us[blk]
                        for gg in range(8):
                            g = blk * 8 + gg
                            for jj in range(4):
                                n_ = g * 4 + jj
                                nc.tensor.matmul(
                                    ps_u[32 * jj: 32 * jj + 32,
                                         gg * OU: (gg + 1) * OU],
                                    xg_nb[:, n_, :], wuv[:, 1, :, n_],
                                    start=False,
                                    stop=(gg == 7 and jj == 3),
                                    tile_position=(0, 32 * jj),
                                    skip_group_check=True,
                                )
                        nc.scalar.activation(
                            hc_sb[:, blk * 512: (blk + 1) * 512],
                            ps_u[:], AF.Tanh)

                    # ---- final combine: h = hc + r*(state - hc) ----
                    t1 = fstg.tile([128, NG * DO], F16, tag="t1")
                    nc.vector.tensor_sub(t1[:], st_grp[:], hc_sb[:])
                    nc.vector.tensor_mul(
                        t1[:].rearrange("p (g o) -> p g o", o=DO),
                        t1[:].rearrange("p (g o) -> p g o", o=DO),
                        zr_sb[:].rearrange("p (g o) -> p g o", o=OG)[:, :, DO:OG],
                    )
                    nc.vector.tensor_add(h_sb[:], t1[:], hc_sb[:])
                    nc.sync.dma_start(h_out[:], h_sb[:])

    nc.finalize()
    return nc


def _get_nc():
    if "nc" not in _CACHE:
        _CACHE["nc"] = _build()
    return _CACHE["nc"]


def kernel(x, state, node_emb, time_emb, gate_w, gate_b, gate_gamma, gate_beta,
           upd_w, upd_b, upd_gamma, upd_beta):
    global LAST_RESULT
    x = np.asarray(x, np.float32)
    state = np.asarray(state, np.float32)
    node_emb = np.asarray(node_emb, np.float32)
    time_emb = np.asarray(time_emb, np.float32)
    gate_w = np.asarray(gate_w, np.float32)
    gate_b = np.asarray(gate_b, np.float32)
    upd_w = np.asarray(upd_w, np.float32)
    upd_b = np.asarray(upd_b, np.float32)

    shared = (
        np.array_equal(np.asarray(gate_gamma), np.ones(D, np.float32))
        and np.array_equal(np.asarray(upd_gamma), np.ones(D, np.float32))
        and np.array_equal(np.asarray(gate_beta), np.zeros(D, np.float32))
        and np.array_equal(np.asarray(upd_beta), np.zeros(D, np.float32))
    )
    if not shared:
        return _np_reference(x, state, node_emb, time_emb, gate_w, gate_b,
                             gate_gamma, gate_beta, upd_w, upd_b, upd_gamma,
                             upd_beta)

    if os.environ.get("BASS_TRACE"):
        _install_prof_shim()

    from concourse.bass_utils import run_bass_kernel_spmd

    nc = _get_nc()
    in_maps = _prep_inmaps(x, state, node_emb, time_emb, gate_w, gate_b,
                           upd_w, upd_b)

    res = run_bass_kernel_spmd(
        nc, in_maps, list(range(NCORES)),
        trace=bool(os.environ.get("BASS_TRACE")),
    )
    LAST_RESULT = res
    return _unpack_h(lambda c: res.results[c]["h_out"])


def _prep_inmaps(x, state, node_emb, time_emb, gate_w, gate_b, upd_w, upd_b):
    inp = np.concatenate([x, state], -1)                      # [B, N, C]
    inpT = np.ascontiguousarray(inp.transpose(2, 0, 1)).astype(np.float16)
    neT = np.ascontiguousarray(node_emb.T).astype(np.float16)  # [D, N]
    teT = np.ascontiguousarray(time_emb.T).astype(np.float16)  # [D, B]
    pg_h = np.ascontiguousarray(
        gate_w.transpose(1, 0, 3, 2).reshape(128, OG * C)).astype(np.float16)
    pu_h = np.ascontiguousarray(
        upd_w.transpose(1, 0, 3, 2).reshape(128, OU * C)).astype(np.float16)
    i32 = np.ascontiguousarray(np.tile(np.eye(32, dtype=np.float16), (1, 4)))

    in_maps = []
    for c in range(NCORES):
        nsl = slice(c * NL, (c + 1) * NL)
        bsl = slice(c * BL, (c + 1) * BL)
        ne2 = np.empty((128, NL), np.float16)
        ne2[0:64] = neT[:, nsl]
        ne2[64:128] = neT[:, nsl]
        inp_cm = np.ascontiguousarray(
            inp[bsl].reshape(BL, 8, 128, C).transpose(2, 0, 1, 3)
            .reshape(128, BL * 8 * C)).astype(np.float16)
        st_grp = np.ascontiguousarray(
            state[:, nsl, :].reshape(B, NG, 4, DO).transpose(2, 0, 1, 3)
            .reshape(128, NG * DO)).astype(np.float16)
        in_maps.append({
            "neT_h": neT,
            "neL2_h": ne2,
            "te_h": np.ascontiguousarray(time_emb.T[:, bsl]).astype(np.float32),
            "teT_h": teT,
            "gb_h": gate_b.astype(np.float16),
            "ub_h": upd_b.astype(np.float16),
            "i32_h": i32,
            "inp_cm_h": inp_cm,
            "inpT_h": np.ascontiguousarray(inpT[:, :, nsl]).reshape(C, B * NL),
            "st_grp_h": st_grp,
            "pg_h": pg_h,
            "pu_h": pu_h,
        })
    return in_maps


def _unpack_h(get_out):
    h = np.empty((B, N, DO), np.float32)
    for c in range(NCORES):
        ho = get_out(c).astype(np.float32).reshape(4, 32, NG, DO)  # [jj,b,g,o]
        h[:, c * NL: (c + 1) * NL, :] = (
            ho.transpose(1, 2, 0, 3).reshape(B, NL, DO)
        )
    return h
